# revision 1
# baseline (speedup 1.0000x reference)
"""Trainium2 Bass kernel for nn_ComplexPatternsNet.

Sharding: L (2048) split 8 ways -> each core processes [B=4, 256] tokens
through embedding gather + RoPE + 3 complex paradox/pattern layers, reduces
its partial `pin` contribution, AllReduces pin across cores, then computes
the tiny final stage and its vocab shard (6656 cols) of the output
projection.  Host pads/permutes out_w, dedups embedding rows per core, and
assembles the final [4, 50257] logits.

Activations live in [feature, token] layout (feature on partitions) as
float32r so every matmul streams at 1 cycle/row on the PE with ~11-bit
mantissa rounding of inputs (fp32 storage, fp32 PSUM accumulation).
"""

import json
import numpy as np

import concourse.bass as bass
import concourse.tile as tile
from concourse import mybir
from concourse.bass_utils import run_bass_kernel_spmd
from concourse.masks import make_identity
from concourse.vector_clock import ScopedClock

F32 = mybir.dt.float32
F32R = mybir.dt.float32r
F16 = mybir.dt.float16
I32 = mybir.dt.int32
AF = mybir.ActivationFunctionType
ALU = mybir.AluOpType

N_CORES = 8
B = 4
L = 2048
LC = L // N_CORES          # 256 positions per core
TOK = B * LC               # 1024 token rows per core
NT = TOK // 128            # 8 gather tiles
D = 512
DC = 256
KB = DC // 128             # 2 feature blocks
NL = 3
NP = 8
TCH = 2                    # token chunks of 512
CHW = TOK // TCH           # 512
V = 50257
VSH = 6656                 # vocab shard per core (13 * 512)
VCH = VSH // 512           # 13
VPAD = VSH * N_CORES       # 53248
SCALE = DC ** -0.5
OUTW_BUFS = 11             # outw prefetch ring (fp16 chunk tiles, 4KB/part)


# ---------------------------------------------------------------------------
# walrus workarounds: this toolchain rejects >1 sem wait per instruction and
# multi-wait kernel-tail drains; split extra waits into EventSemaphore insts.
# ---------------------------------------------------------------------------

def _split_multiwait_json(d: dict) -> dict:
    ctr = 0
    for fn in d.get("functions", []):
        for bb in fn.get("blocks", []):
            out = []
            for inst in bb.get("instructions", []):
                si = inst.get("sync_info")
                waits = (si or {}).get("on_wait") or []
                if len(waits) > 1:
                    for w in waits[:-1]:
                        out.append({
                            "opcode": "EventSemaphore",
                            "name": f"wsplit-{ctr}",
                            "engine": inst["engine"],
                            "ins": [],
                            "outs": [],
                            "sync_info": {"on_update": [], "on_wait": [w]},
                            "debug": inst.get("debug"),
                        })
                        ctr += 1
                    si["on_wait"] = [waits[-1]]
                out.append(inst)
            bb["instructions"] = out
    return d


class SplitWaitBass(bass.Bass):
    def to_json_bytes(self) -> bytes:
        d = json.loads(super().to_json_bytes())
        d = _split_multiwait_json(d)
        return json.dumps(d).encode()


class SplitDrainTileContext(tile.TileContext):
    def _drain_and_barrier(self, tick_clock, wait_clock):
        nc = self.nc
        scratch = nc.sync.nop()
        wait_clock.add_sem_waits(
            scratch.ins, ScopedClock({None: tick_clock.global_clock})
        )
        si = scratch.ins.sync_info
        waits = list(si.on_wait) if si is not None else []
        if si is not None:
            si.on_wait = []
        assert self.sems is not None
        by_num = {h.num: h for h in self.sems.allocated().values()}
        for w in waits:
            h = by_num.get(w.id)
            assert h is not None, f"unmapped drain wait {w}"
            nc.sync.wait_ge(h, w.wait_value)
        nc.sync.drain()
        nc.all_engine_barrier(sem_only=True)
        popped = nc._tile_sem_poison_stack.pop()
        assert popped is self._sem_poison
        nc.clear_and_free_semaphores(list(self.sems.allocated().values()))
        nc.all_engine_barrier(sem_only=True)


# ---------------------------------------------------------------------------
# device kernel
# ---------------------------------------------------------------------------

def build_nc():
    nc = SplitWaitBass(num_devices=N_CORES)

    emb_t = nc.dram_tensor("emb_t", [TOK, D], F32, kind="ExternalInput")
    tok_idx = nc.dram_tensor("tok_idx", [NT, 128, 1], I32, kind="ExternalInput")
    rope_cos = nc.dram_tensor("rope_cos", [NT, 128, DC], F32, kind="ExternalInput")
    rope_sin = nc.dram_tensor("rope_sin", [NT, 128, DC], F32, kind="ExternalInput")
    lw = nc.dram_tensor("lw", [NL, 128, 36 * 128], F32R, kind="ExternalInput")
    lb = nc.dram_tensor("lb", [128, 36], F32, kind="ExternalInput")
    patT = nc.dram_tensor("patT", [128, NL * 2 * KB * NP], F32R, kind="ExternalInput")
    patM = nc.dram_tensor("patM", [NP, NL * 2 * KB * 128], F32R, kind="ExternalInput")
    pw = nc.dram_tensor("pw", [128, 2 * 3 * KB * KB * 128], F32R, kind="ExternalInput")
    pb = nc.dram_tensor("pb", [128, 8], F32, kind="ExternalInput")
    ppT = nc.dram_tensor("ppT", [128, 2 * KB * NP], F32R, kind="ExternalInput")
    ppM = nc.dram_tensor("ppM", [NP, 2 * KB * 128], F32R, kind="ExternalInput")
    outw = nc.dram_tensor("outw", [VCH, 128, 2 * KB * 512], F16, kind="ExternalInput")

    logits = nc.dram_tensor("logits", [B, VSH], F32, kind="ExternalOutput")

    cc_in = nc.dram_tensor("cc_in", [128, 16], F32)
    cc_out = nc.dram_tensor("cc_out", [128, 16], F32, addr_space="Shared")

    # lw column layout: col = (((mat*3 + var)*KB + kblk)*KB + mblk)*128 + mcol
    def lwcol(mat, var, kblk, mblk):
        return (((mat * 3 + var) * KB + kblk) * KB + mblk) * 128

    def lbcol(lay, mat, var, mblk):
        return ((lay * 3 + mat) * 2 + var) * 2 + mblk

    def patTcol(lay, var, kblk):
        return ((lay * 2 + var) * KB + kblk) * NP

    def patMcol(lay, var, mblk):
        return ((lay * 2 + var) * KB + mblk) * 128

    def pwcol(mat, var, kblk, mblk):
        return (((mat * 3 + var) * KB + kblk) * KB + mblk) * 128

    def pbcol(mat, var, mblk):
        return (mat * 2 + var) * 2 + mblk

    def ppTcol(var, kblk):
        return (var * KB + kblk) * NP

    def ppMcol(var, mblk):
        return (var * KB + mblk) * 128

    with SplitDrainTileContext(nc) as tc:
        with (
            tc.tile_pool(name="wres", bufs=1) as wres,
            tc.tile_pool(name="lwp", bufs=3) as lwp,
            tc.tile_pool(name="gp", bufs=2) as gp,
            tc.tile_pool(name="actp", bufs=1) as actp,
            tc.tile_pool(name="dp", bufs=2) as dp,
            tc.tile_pool(name="genp", bufs=2) as genp,
            tc.tile_pool(name="smp", bufs=2) as smp,
            tc.tile_pool(name="op", bufs=OUTW_BUFS) as op,
            tc.tile_pool(name="lop", bufs=2) as lop,
            tc.tile_pool(name="psA", bufs=4, space="PSUM") as psA,
            tc.tile_pool(name="psS", bufs=2, space="PSUM") as psS,
            tc.tile_pool(name="psF", bufs=2, space="PSUM") as psF,
        ):
            # ---- resident constants/weights ----
            identf = wres.tile([128, 128], F32)
            make_identity(nc, identf[:])
            ident = wres.tile([128, 128], F32R)
            nc.vector.tensor_copy(ident[:], identf[:])
            i07 = wres.tile([128, 128], F32R)
            nc.vector.tensor_scalar_mul(i07[:], identf[:], 0.7)
            ineg = wres.tile([128, 128], F32R)
            nc.vector.tensor_scalar_mul(ineg[:], identf[:], -1.0)
            ones8f = wres.tile([NP, 1], F32)
            nc.vector.memset(ones8f[:], 1.0)
            ones8 = wres.tile([NP, 1], F32R)
            nc.vector.tensor_copy(ones8[:], ones8f[:])
            ones18f = wres.tile([1, NP], F32)
            nc.vector.memset(ones18f[:], 1.0)
            ones18 = wres.tile([1, NP], F32R)
            nc.vector.tensor_copy(ones18[:], ones18f[:])

            lb_sb = wres.tile([128, 36], F32)
            nc.sync.dma_start(lb_sb[:], lb[:])
            patT_sb = wres.tile([128, NL * 2 * KB * NP], F32R)
            nc.sync.dma_start(patT_sb[:], patT[:])
            patM_sb = wres.tile([NP, NL * 2 * KB * 128], F32R)
            nc.sync.dma_start(patM_sb[:], patM[:])
            pw_sb = wres.tile([128, 2 * 3 * KB * KB * 128], F32R)
            nc.sync.dma_start(pw_sb[:], pw[:])
            pb_sb = wres.tile([128, 8], F32)
            nc.sync.dma_start(pb_sb[:], pb[:])
            ppT_sb = wres.tile([128, 2 * KB * NP], F32R)
            nc.sync.dma_start(ppT_sb[:], ppT[:])
            ppM_sb = wres.tile([NP, 2 * KB * 128], F32R)
            nc.sync.dma_start(ppM_sb[:], ppM[:])

            # ---- phase A: gather + rope + transpose into [feat, tok] ----
            # cur[part][kblk] tiles [128, TOK]
            cur = [[genp.tile([128, TOK], F32R, tag=f"gen{p}{k}", name=f"cur{p}{k}")
                    for k in range(KB)] for p in range(2)]
            for t in range(NT):
                tokt = gp.tile([128, 1], I32, tag="tok")
                nc.sync.dma_start(tokt[:], tok_idx[t])
                xt = gp.tile([128, D], F32, tag="x")
                nc.gpsimd.indirect_dma_start(
                    out=xt[:], out_offset=None, in_=emb_t[:],
                    in_offset=bass.IndirectOffsetOnAxis(ap=tokt[:, :1], axis=0),
                )
                cost = gp.tile([128, DC], F32, tag="cos")
                nc.sync.dma_start(cost[:], rope_cos[t])
                sint = gp.tile([128, DC], F32, tag="sin")
                nc.sync.dma_start(sint[:], rope_sin[t])
                xv = xt[:].rearrange("p (f two) -> p f two", two=2)
                xr, xi = xv[:, :, 0], xv[:, :, 1]
                t1 = gp.tile([128, DC], F32, tag="rt1")
                t2 = gp.tile([128, DC], F32, tag="rt2")
                ctr = gp.tile([128, DC], F32R, tag="ctr")
                cti = gp.tile([128, DC], F32R, tag="cti")
                nc.vector.tensor_tensor(t1[:], xr, cost[:], op=ALU.mult)
                nc.vector.tensor_tensor(t2[:], xi, sint[:], op=ALU.mult)
                nc.vector.tensor_tensor(ctr[:], t1[:], t2[:], op=ALU.subtract)
                nc.vector.tensor_tensor(t1[:], xr, sint[:], op=ALU.mult)
                nc.vector.tensor_tensor(t2[:], xi, cost[:], op=ALU.mult)
                nc.vector.tensor_tensor(cti[:], t1[:], t2[:], op=ALU.add)
                for part, src in ((0, ctr), (1, cti)):
                    for kb in range(KB):
                        pst = psF.tile([128, 128], F32R, tag="fin")
                        nc.tensor.transpose(
                            pst[:], src[:, kb * 128:(kb + 1) * 128], ident[:])
                        nc.scalar.activation(
                            cur[part][kb][:, t * 128:(t + 1) * 128], pst[:], AF.Copy)

            # ---- outw prefetch ring (after prep loads so those win priority) ----
            outw_t = {}
            dma_engines = [nc.sync, nc.scalar, nc.gpsimd]
            for ch in range(VCH):
                t = op.tile([128, 2 * KB * 512], F16, tag="outw")
                dma_engines[ch % 3].dma_start(t[:], outw[ch])
                outw_t[ch] = t

            # ---- layers ----
            pen_sums = [[actp.tile([128, 16], F32, tag=f"psum{p}{m}", name=f"pensums{p}{m}")
                         for m in range(KB)] for p in range(2)]

            for lay in range(NL):
                lwt = {}
                for mat in range(3):
                    t = lwp.tile([128, 12 * 128], F32R, tag="lw")
                    nc.sync.dma_start(
                        t[:], lw[lay, :, mat * 12 * 128:(mat + 1) * 12 * 128])
                    lwt[mat] = t

                def wsl(mat, var, kblk, mblk):
                    c = lwcol(mat, var, kblk, mblk) - mat * 12 * 128
                    return lwt[mat][:, c:c + 128]

                # hl = cur @ Wp + bp   (per-chunk tiles for finer rotation)
                hl = {}
                for ch in range(TCH):
                    for part in range(2):
                        terms = ([(0, 0), (2, 1)] if part == 0 else [(1, 0), (0, 1)])
                        for mb in range(KB):
                            ps = psA.tile([128, CHW], F32, tag="mm")
                            first = True
                            for var, apart in terms:
                                for kb in range(KB):
                                    nc.tensor.matmul(
                                        ps[:], wsl(0, var, kb, mb),
                                        cur[apart][kb][:, ch * CHW:(ch + 1) * CHW],
                                        start=first, stop=(var, apart, kb) == (terms[1][0], terms[1][1], KB - 1))
                                    first = False
                            bcol = lbcol(lay, 0, part, mb)
                            hlt = actp.tile([128, CHW], F32R, tag=f"hl{part}{mb}{ch}",
                                            name=f"hl{lay}_{part}{mb}{ch}", bufs=1)
                            nc.vector.tensor_scalar_add(
                                hlt[:], ps[:], lb_sb[:, bcol:bcol + 1])
                            hl[(part, mb, ch)] = hlt

                # d = cur @ (Wp@Ws) + (bp@Ws + bs) - hl  (W' host-precomputed so
                # these matmuls don't wait on the hl evacuation)
                h = {}
                for ch in range(TCH):
                    for mb in range(KB):
                        dd = {}
                        for part in range(2):
                            terms = ([(0, 0), (2, 1)] if part == 0 else [(1, 0), (0, 1)])
                            ps = psA.tile([128, CHW], F32, tag="mm")
                            first = True
                            for var, apart in terms:
                                for kb in range(KB):
                                    nc.tensor.matmul(
                                        ps[:], wsl(1, var, kb, mb),
                                        cur[apart][kb][:, ch * CHW:(ch + 1) * CHW],
                                        start=first, stop=(var, apart, kb) == (terms[1][0], terms[1][1], KB - 1))
                                    first = False
                            dt_ = dp.tile([128, CHW], F32, tag="d")
                            bcol = lbcol(lay, 1, part, mb)
                            nc.scalar.activation(
                                dt_[:], ps[:], AF.Identity,
                                bias=lb_sb[:, bcol:bcol + 1])
                            dd[part] = dt_
                        sq1 = dp.tile([128, CHW], F32, tag="sq1")
                        sq2 = dp.tile([128, CHW], F32, tag="sq2")
                        nc.vector.tensor_tensor(sq1[:], dd[0][:], dd[0][:], op=ALU.mult)
                        nc.vector.tensor_tensor(sq2[:], dd[1][:], dd[1][:], op=ALU.mult)
                        nc.vector.tensor_tensor(sq1[:], sq1[:], sq2[:], op=ALU.add)
                        nc.scalar.activation(sq2[:], sq1[:], AF.Sqrt)
                        gch = actp.tile([128, CHW], F32, tag=f"g{mb}{ch}",
                                        name=f"g{lay}_{mb}{ch}", bufs=1)
                        nc.scalar.activation(gch[:], sq2[:], AF.Sigmoid)
                        for part in range(2):
                            ht = actp.tile([128, CHW], F32R, tag=f"h{part}{mb}{ch}",
                                           name=f"h{lay}_{part}{mb}{ch}", bufs=1)
                            nc.vector.tensor_tensor(
                                ht[:], hl[(part, mb, ch)][:], gch[:], op=ALU.mult)
                            h[(part, mb, ch)] = ht

                # pattern attention: scores -> softmax -> att (0.3 folded),
                # mixed = 0.7 h + att_norm
                e_n = {}
                for ch in range(TCH):
                    pse = psS.tile([NP, CHW], F32, tag="sc")
                    first = True
                    for var in range(2):
                        for kb in range(KB):
                            c = patTcol(lay, var, kb)
                            nc.tensor.matmul(
                                pse[:], patT_sb[:, c:c + NP],
                                h[(var, kb, ch)][:],
                                start=first, stop=(var, kb) == (1, KB - 1))
                            first = False
                    et = dp.tile([NP, CHW], F32R, tag="e")
                    nc.scalar.activation(et[:], pse[:], AF.Exp)
                    pssum = psS.tile([1, CHW], F32, tag="sc")
                    nc.tensor.matmul(pssum[:], ones8[:], et[:], start=True, stop=True)
                    rcp = dp.tile([1, CHW], F32R, tag="rcp")
                    with nc.allow_low_precision(reason="f32r is fp32 storage"):
                        nc.vector.reciprocal(rcp[:], pssum[:])
                    psb8 = psS.tile([NP, CHW], F32, tag="sc")
                    nc.tensor.matmul(psb8[:], ones18[:], rcp[:], start=True, stop=True)
                    ent = dp.tile([NP, CHW], F32R, tag="en2")
                    nc.vector.tensor_tensor(ent[:], et[:], psb8[:], op=ALU.mult)
                    e_n[ch] = ent

                mixed = [[genp.tile([128, TOK], F32R, tag=f"gen{p}{k}", name=f"mixed{lay}_{p}{k}")
                          for k in range(KB)] for p in range(2)]
                for ch in range(TCH):
                    for part in range(2):
                        for mb in range(KB):
                            ps = psA.tile([128, CHW], F32, tag="mm")
                            c = patMcol(lay, part, mb)
                            nc.tensor.matmul(
                                ps[:], patM_sb[:, c:c + 128],
                                e_n[ch][:],
                                start=True, stop=False)
                            nc.tensor.matmul(
                                ps[:], i07[:],
                                h[(part, mb, ch)][:],
                                start=False, stop=True)
                            nc.vector.tensor_copy(
                                mixed[part][mb][:, ch * CHW:(ch + 1) * CHW], ps[:])

                # pen = mixed @ Wpen + bpen via linearity: batch-sum mixed
                # first (ACT accum), then N=4 matmuls on the sums.
                msum = [[smp.tile([128, B], F32R, tag=f"ms{p}{m}",
                                  name=f"msum{lay}_{p}{m}")
                         for m in range(KB)] for p in range(2)]
                with nc.allow_low_precision(reason="f32r is fp32 storage"):
                    for part in range(2):
                        for mb in range(KB):
                            for b in range(B):
                                scr = dp.tile([128, LC], F32, tag="scr")
                                nc.scalar.activation(
                                    scr[:], mixed[part][mb][:, b * LC:(b + 1) * LC],
                                    AF.Copy,
                                    accum_out=msum[part][mb][:, b:b + 1])
                for part in range(2):
                    terms = ([(0, 0), (2, 1)] if part == 0 else [(1, 0), (0, 1)])
                    for mb in range(KB):
                        ps = psF.tile([128, B], F32, tag="fin")
                        first = True
                        for var, apart in terms:
                            for kb in range(KB):
                                nc.tensor.matmul(
                                    ps[:], wsl(2, var, kb, mb), msum[apart][kb][:],
                                    start=first, stop=(var, apart, kb) == (terms[1][0], terms[1][1], KB - 1))
                                first = False
                        bcol = lbcol(lay, 2, part, mb)
                        pview = pen_sums[part][mb][:].rearrange(
                            "p (b w) -> p b w", w=4)[:, :, lay]
                        nc.vector.tensor_scalar_add(pview, ps[:], lb_sb[:, bcol:bcol + 1])
                        if lay == NL - 1:
                            cview = pen_sums[part][mb][:].rearrange(
                                "p (b w) -> p b w", w=4)[:, :, 3]
                            nc.vector.tensor_copy(cview, msum[part][mb][:])
                cur = mixed

            # ---- pin partial = (sum pen + sum cur) / L ----
            pinp = smp.tile([128, 16], F32, tag="pinp")
            for part in range(2):
                for mb in range(KB):
                    red = smp.tile([128, 4], F32, tag="red")
                    nc.vector.tensor_reduce(
                        red[:], pen_sums[part][mb][:].rearrange("p (b w) -> p b w", w=4),
                        axis=mybir.AxisListType.X, op=ALU.add)
                    col = (part * KB + mb) * 4
                    nc.vector.tensor_scalar_mul(
                        pinp[:, col:col + 4], red[:], 1.0 / L)
            nc.sync.dma_start(cc_in[:], pinp[:])
            nc.gpsimd.collective_compute(
                "AllReduce", ALU.add,
                replica_groups=[list(range(N_CORES))],
                ins=[cc_in[:].opt()], outs=[cc_out[:].opt()],
            )
            pin = smp.tile([128, 16], F32R, tag="pinr")
            nc.gpsimd.dma_start(pin[:], cc_out[:])

            def pincol(part, kb):
                return (part * KB + kb) * 4

            # ---- final paradox (pw_process / pw_self) ----
            hl2 = [[None] * KB for _ in range(2)]
            for part in range(2):
                terms = ([(0, 0), (2, 1)] if part == 0 else [(1, 0), (0, 1)])
                for mb in range(KB):
                    ps = psF.tile([128, B], F32, tag="fin")
                    first = True
                    for var, apart in terms:
                        for kb in range(KB):
                            c = pwcol(0, var, kb, mb)
                            nc.tensor.matmul(
                                ps[:], pw_sb[:, c:c + 128],
                                pin[:, pincol(apart, kb):pincol(apart, kb) + 4],
                                start=first, stop=(var, apart, kb) == (terms[1][0], terms[1][1], KB - 1))
                            first = False
                    t = smp.tile([128, B], F32R, tag=f"hl2{part}{mb}")
                    c = pbcol(0, part, mb)
                    nc.vector.tensor_scalar_add(t[:], ps[:], pb_sb[:, c:c + 1])
                    hl2[part][mb] = t
            g2 = []
            dd2 = [[None] * KB for _ in range(2)]
            for mb in range(KB):
                for part in range(2):
                    terms = ([(0, 0), (2, 1)] if part == 0 else [(1, 0), (0, 1)])
                    ps = psF.tile([128, B], F32, tag="fin")
                    first = True
                    for var, apart in terms:
                        for kb in range(KB):
                            c = pwcol(1, var, kb, mb)
                            nc.tensor.matmul(
                                ps[:], pw_sb[:, c:c + 128],
                                pin[:, pincol(apart, kb):pincol(apart, kb) + 4],
                                start=first, stop=(var, apart, kb) == (terms[1][0], terms[1][1], KB - 1))
                            first = False
                    t = smp.tile([128, B], F32, tag=f"dd2{part}{mb}")
                    c = pbcol(1, part, mb)
                    nc.vector.tensor_scalar_add(t[:], ps[:], pb_sb[:, c:c + 1])
                    dd2[part][mb] = t
                s1 = smp.tile([128, B], F32, tag="s1")
                s2 = smp.tile([128, B], F32, tag="s2")
                nc.vector.tensor_tensor(s1[:], dd2[0][mb][:], dd2[0][mb][:], op=ALU.mult)
                nc.vector.tensor_tensor(s2[:], dd2[1][mb][:], dd2[1][mb][:], op=ALU.mult)
                nc.vector.tensor_tensor(s1[:], s1[:], s2[:], op=ALU.add)
                nc.scalar.activation(s2[:], s1[:], AF.Sqrt)
                gt = smp.tile([128, B], F32, tag=f"g2{mb}")
                nc.scalar.activation(gt[:], s2[:], AF.Sigmoid)
                g2.append(gt)
            h2 = [[None] * KB for _ in range(2)]
            for part in range(2):
                for mb in range(KB):
                    t = smp.tile([128, B], F32R, tag=f"h2{part}{mb}")
                    nc.vector.tensor_tensor(t[:], hl2[part][mb][:], g2[mb][:], op=ALU.mult)
                    h2[part][mb] = t

            # ---- attn2 ----
            ps2 = psF.tile([B, NP], F32, tag="fin")
            first = True
            for var in range(2):
                for kb in range(KB):
                    c = ppTcol(var, kb)
                    nc.tensor.matmul(ps2[:], h2[var][kb][:], ppT_sb[:, c:c + NP],
                                     start=first, stop=(var, kb) == (1, KB - 1))
                    first = False
            e2 = smp.tile([B, NP], F32, tag="e2")
            se = smp.tile([B, 1], F32, tag="se")
            nc.scalar.activation(e2[:], ps2[:], AF.Exp, accum_out=se[:])
            rcp2 = smp.tile([B, 1], F32, tag="rcp2")
            with nc.allow_low_precision(reason="tiny"):
                nc.vector.reciprocal(rcp2[:], se[:])
            attw = smp.tile([B, NP], F32R, tag="attw")
            nc.vector.tensor_scalar_mul(attw[:], e2[:], rcp2[:, :1])
            psw = psF.tile([NP, B], F32R, tag="fin")
            nc.tensor.transpose(psw[:], attw[:], ident[:B, :B])
            attwT = smp.tile([NP, B], F32R, tag="attwT")
            nc.vector.tensor_copy(attwT[:], psw[:])

            m2 = []
            for part in range(2):
                for mb in range(KB):
                    ps = psF.tile([128, B], F32, tag="fin")
                    c = ppMcol(part, mb)
                    nc.tensor.matmul(ps[:], ppM_sb[:, c:c + 128], attwT[:],
                                     start=True, stop=False)
                    nc.tensor.matmul(ps[:], i07[:], h2[part][mb][:],
                                     start=False, stop=True)
                    t = smp.tile([128, B], F16, tag=f"m2{part}{mb}")
                    nc.vector.tensor_copy(t[:], ps[:])
                    m2.append(t)

            # ---- vocab projection ----
            for ch in range(VCH):
                ps = psF.tile([B, 512], F32, tag="fin")
                for kb in range(2 * KB):
                    nc.tensor.matmul(ps[:], m2[kb][:],
                                     outw_t[ch][:, kb * 512:(kb + 1) * 512],
                                     start=(kb == 0), stop=(kb == 2 * KB - 1))
                lo = lop.tile([B, 512], F32, tag="lo")
                nc.vector.tensor_copy(lo[:], ps[:])
                nc.sync.dma_start(logits[:, ch * 512:(ch + 1) * 512], lo[:])

    return nc


_NC_CACHE = None


def _get_nc():
    global _NC_CACHE
    if _NC_CACHE is None:
        _NC_CACHE = build_nc()
    return _NC_CACHE


# ---------------------------------------------------------------------------
# host side
# ---------------------------------------------------------------------------

def _prep_core_inputs(c, tokens, emb, lw_process, lb_process, lw_self, lb_self,
                      lw_pen, lb_pen, patterns, pw_process, pb_process, pw_self,
                      pb_self, p_patterns, out_w_perm):
    f32 = np.float32
    toks = np.ascontiguousarray(tokens[:, c * LC:(c + 1) * LC]).reshape(-1)
    uniq, inv = np.unique(toks, return_inverse=True)
    emb_t = np.zeros((TOK, D), f32)
    emb_t[:len(uniq)] = emb[uniq]
    tok_idx = inv.astype(np.int32).reshape(NT, 128, 1)

    pos = (np.arange(LC, dtype=f32) + c * LC)
    freqs = (10000.0 ** (-np.arange(DC, dtype=f32) / DC))
    ang = pos[:, None] * freqs[None, :]            # [LC, DC]
    cosl = np.cos(ang).astype(f32)
    sinl = np.sin(ang).astype(f32)
    rope_cos = np.tile(cosl, (B, 1)).reshape(NT, 128, DC)
    rope_sin = np.tile(sinl, (B, 1)).reshape(NT, 128, DC)

    lw_arr = np.zeros((NL, 128, 36 * 128), f32)
    lb_arr = np.zeros((128, 36), f32)
    mats_w = [lw_process, lw_self, lw_pen]
    mats_b = [lb_process, lb_self, lb_pen]
    for lay in range(NL):
        Wp_c = (lw_process[lay, :, :, 0] + 1j * lw_process[lay, :, :, 1]).astype(np.complex128)
        Ws_c = (lw_self[lay, :, :, 0] + 1j * lw_self[lay, :, :, 1]).astype(np.complex128)
        bp_c = (lb_process[lay, :, 0] + 1j * lb_process[lay, :, 1]).astype(np.complex128)
        bs_c = (lb_self[lay, :, 0] + 1j * lb_self[lay, :, 1]).astype(np.complex128)
        WsI = Ws_c - np.eye(DC, dtype=np.complex128)
        Wprod = Wp_c @ WsI
        bprod = bp_c @ WsI + bs_c
        for mat in range(3):
            if mat == 1:
                Wr = Wprod.real.astype(f32)
                Wi = Wprod.imag.astype(f32)
            else:
                Wr = mats_w[mat][lay, :, :, 0]
                Wi = mats_w[mat][lay, :, :, 1]
            for var, Wv in enumerate((Wr, Wi, -Wi)):
                for kb in range(KB):
                    for mb in range(KB):
                        col = (((mat * 3 + var) * KB + kb) * KB + mb) * 128
                        lw_arr[lay, :, col:col + 128] = \
                            Wv[kb * 128:(kb + 1) * 128, mb * 128:(mb + 1) * 128]
            for var in range(2):
                if mat == 1:
                    bv = (bprod.real if var == 0 else bprod.imag).astype(f32)
                else:
                    bv = mats_b[mat][lay, :, var]
                if mat == 2:
                    # pen bias is applied to a per-batch token sum
                    bv = bv * LC
                for mb in range(KB):
                    lb_arr[:, ((lay * 3 + mat) * 2 + var) * 2 + mb] = \
                        bv[mb * 128:(mb + 1) * 128]

    patT_arr = np.zeros((128, NL * 2 * KB * NP), f32)
    patM_arr = np.zeros((NP, NL * 2 * KB * 128), f32)
    for lay in range(NL):
        for var in range(2):
            Pv = patterns[lay, :, :, var]           # [NP, DC]
            for kb in range(KB):
                patT_arr[:, ((lay * 2 + var) * KB + kb) * NP:
                            ((lay * 2 + var) * KB + kb) * NP + NP] = \
                    (Pv[:, kb * 128:(kb + 1) * 128] * SCALE).T
            for mb in range(KB):
                patM_arr[:, ((lay * 2 + var) * KB + mb) * 128:
                            ((lay * 2 + var) * KB + mb) * 128 + 128] = \
                    Pv[:, mb * 128:(mb + 1) * 128] * 0.3

    pw_arr = np.zeros((128, 2 * 3 * KB * KB * 128), f32)
    pb_arr = np.zeros((128, 8), f32)
    pwp_c = (pw_process[:, :, 0] + 1j * pw_process[:, :, 1]).astype(np.complex128)
    pws_c = (pw_self[:, :, 0] + 1j * pw_self[:, :, 1]).astype(np.complex128)
    pbp_c = (pb_process[:, 0] + 1j * pb_process[:, 1]).astype(np.complex128)
    pbs_c = (pb_self[:, 0] + 1j * pb_self[:, 1]).astype(np.complex128)
    pWsI = pws_c - np.eye(DC, dtype=np.complex128)
    pWq = pwp_c @ pWsI
    pbq = pbp_c @ pWsI + pbs_c
    for mat in range(2):
        if mat == 0:
            Wr, Wi = pw_process[:, :, 0], pw_process[:, :, 1]
            br, bi = pb_process[:, 0], pb_process[:, 1]
        else:
            Wr = pWq.real.astype(f32); Wi = pWq.imag.astype(f32)
            br = pbq.real.astype(f32); bi = pbq.imag.astype(f32)
        for var, Wv in enumerate((Wr, Wi, -Wi)):
            for kb in range(KB):
                for mb in range(KB):
                    col = (((mat * 3 + var) * KB + kb) * KB + mb) * 128
                    pw_arr[:, col:col + 128] = \
                        Wv[kb * 128:(kb + 1) * 128, mb * 128:(mb + 1) * 128]
        for var in range(2):
            bv = br if var == 0 else bi
            for mb in range(KB):
                pb_arr[:, (mat * 2 + var) * 2 + mb] = \
                    bv[mb * 128:(mb + 1) * 128]

    ppT_arr = np.zeros((128, 2 * KB * NP), f32)
    ppM_arr = np.zeros((NP, 2 * KB * 128), f32)
    for var in range(2):
        Pv = p_patterns[:, :, var]
        for kb in range(KB):
            ppT_arr[:, (var * KB + kb) * NP:(var * KB + kb) * NP + NP] = \
                (Pv[:, kb * 128:(kb + 1) * 128] * SCALE).T
        for mb in range(KB):
            ppM_arr[:, (var * KB + mb) * 128:(var * KB + mb) * 128 + 128] = \
                Pv[:, mb * 128:(mb + 1) * 128] * 0.3

    ow = out_w_perm[:, c * VSH:(c + 1) * VSH]       # [512, VSH]
    outw_arr = np.ascontiguousarray(
        ow.reshape(2 * KB, 128, VCH, 512).transpose(2, 1, 0, 3)
        .reshape(VCH, 128, 2 * KB * 512)).astype(np.float16)

    return {
        "emb_t": emb_t, "tok_idx": tok_idx,
        "rope_cos": np.ascontiguousarray(rope_cos),
        "rope_sin": np.ascontiguousarray(rope_sin),
        "lw": lw_arr, "lb": lb_arr, "patT": patT_arr, "patM": patM_arr,
        "pw": pw_arr, "pb": pb_arr, "ppT": ppT_arr, "ppM": ppM_arr,
        "outw": outw_arr,
    }


def kernel(tokens, emb, lw_process, lb_process, lw_self, lb_self, lw_pen,
           lb_pen, patterns, pw_process, pb_process, pw_self, pb_self,
           p_patterns, out_w, out_b, _trace=False):
    tokens = np.asarray(tokens)
    args = [np.asarray(a, np.float32) for a in
            (emb, lw_process, lb_process, lw_self, lb_self, lw_pen, lb_pen,
             patterns, pw_process, pb_process, pw_self, pb_self, p_patterns)]
    out_w = np.asarray(out_w, np.float32)
    out_b = np.asarray(out_b, np.float32)

    # permute rows of out_w to the device feats layout and pad the vocab
    perm = 2 * (np.arange(D) % DC) + (np.arange(D) // DC)
    ow_pad = np.zeros((D, VPAD), np.float32)
    ow_pad[:, :V] = out_w[perm]

    in_maps = [
        _prep_core_inputs(c, tokens, *args, ow_pad) for c in range(N_CORES)
    ]
    nc = _get_nc()
    res = run_bass_kernel_spmd(
        nc, in_maps, core_ids=list(range(N_CORES)), trace=_trace)
    logits = np.concatenate(
        [res.results[c]["logits"] for c in range(N_CORES)], axis=1)[:, :V]
    out = logits + out_b[None, :]
    if _trace:
        kernel.last_results = res
    return out.astype(np.float32)

